# revision 25
# baseline (speedup 1.0000x reference)
"""MinkowskiSwitchNorm Trainium2 kernel (8 NeuronCores, Bass/Tile).

Strategy: host sorts points by segment id so that every 8192-point chunk is
single-segment, then ships x TRANSPOSED per core as [128, 65536] bf16 where
partition p = (half h = p//64, channel c = p%64) and free column j is point
h*65536 + j.  Free sub-slice m (width 4096) therefore lies inside chunk
m//2 (partitions 0:64) and chunk 8+m//2 (partitions 64:128).

Pass 1 computes per-(chunk,channel) sums: sum(x) via tensor_scalar+accum_out
on the vector engine, sum(x^2) via Square-activation+accum_out on the scalar
engine (with a few sub-slices on the vector engine's scalar_tensor_tensor to
balance load).  A PE transpose plus tiny one-hot selector matmuls aggregate
them into per-segment [2B, C] partials, combined across cores by a 4KB
AllReduce.  On-chip stats produce per-segment tables A = inv_std*w,
D = b - mean*A; four small matmuls expand them to per-partition per-slice
vectors, and pass 2 is a single fused tensor_scalar out = x*A + D per
sub-slice, written back as bf16 and upcast + unsorted on the host.  x stays
resident in SBUF between the passes, so HBM traffic is 16MB in + 16MB out
per core.
"""

import numpy as np
import ml_dtypes
from contextlib import ExitStack

import concourse.bass as bass
import concourse.tile as tile
from concourse import bacc, mybir
from concourse.bass_utils import run_bass_kernel_spmd

NCORES = 8
B = 8            # segments
C = 64           # channels
NTOT = 1_000_000
P = 128
NLP = 131072             # padded rows per core
HALF = NLP // 2          # 65536 free columns
CHP = 8192               # points per chunk
NCH = NLP // CHP         # chunks per core = 16
NSL = 16                 # free sub-slices per core
SL = HALF // NSL         # sub-slice width = 4096
TOTCH = NCORES * NCH     # 128 chunks globally
EPS = 1e-5
F32 = mybir.dt.float32
BF16 = mybir.dt.bfloat16

# engine assignment for sum(x^2) per sub-slice
ACT_SQ = frozenset([0, 1, 2, 3, 4, 5, 6, 7, 8, 9, 14, 15])  # rest: DVE STT
NCHH = NCH // 2          # chunk pairs per core = 8 (sum(x) granularity)
NSC = NCHH + NSL         # S columns: 8 chunk sum(x) + 16 slice sum(x^2)

_CACHE = {}


def _build():
    nc = bacc.Bacc("TRN2", target_bir_lowering=False, debug=False,
                   num_devices=NCORES)

    xt_i = nc.dram_tensor("xt", [P, HALF], BF16, kind="ExternalInput").ap()
    selagg_i = nc.dram_tensor("selagg", [NSC, 32], BF16,
                              kind="ExternalInput").ap()
    identb_i = nc.dram_tensor("identb", [P, P], BF16,
                              kind="ExternalInput").ap()
    selad_i = nc.dram_tensor("selad", [B, 2 * NSL], F32,
                             kind="ExternalInput").ap()
    ident_i = nc.dram_tensor("ident", [P, P], F32, kind="ExternalInput").ap()
    w_i = nc.dram_tensor("wt", [1, C], F32, kind="ExternalInput").ap()
    b_i = nc.dram_tensor("bs", [1, C], F32, kind="ExternalInput").ap()
    hs_i = nc.dram_tensor("hs", [B, 8], F32, kind="ExternalInput").ap()
    c82_i = nc.dram_tensor("c82", [B, 2], F32, kind="ExternalInput").ap()
    out_o = nc.dram_tensor("out", [P, HALF], BF16, kind="ExternalOutput").ap()

    cc_in = nc.dram_tensor("cc_in", [16, C], F32)
    cc_out = nc.dram_tensor("cc_out", [16, C], F32, addr_space="Shared")

    with ExitStack() as ctx:
        tc = ctx.enter_context(tile.TileContext(nc))
        xpool = ctx.enter_context(tc.tile_pool(name="xres", bufs=NSL))
        opool = ctx.enter_context(tc.tile_pool(name="outb", bufs=2))
        dpool = ctx.enter_context(tc.tile_pool(name="dumb", bufs=1))
        singles = ctx.enter_context(tc.tile_pool(name="singles", bufs=1))
        psum = ctx.enter_context(tc.tile_pool(name="ps", bufs=1, space="PSUM"))

        ident = singles.tile([P, P], F32)
        nc.sync.dma_start(out=ident[:], in_=ident_i[:])
        identb = singles.tile([P, P], BF16)
        nc.sync.dma_start(out=identb[:], in_=identb_i[:])
        selagg = singles.tile([NSC, 32], BF16)
        nc.sync.dma_start(out=selagg[:], in_=selagg_i[:])
        selad = singles.tile([B, 2 * NSL], F32)
        nc.sync.dma_start(out=selad[:], in_=selad_i[:])
        hs = singles.tile([B, 8], F32)
        nc.sync.dma_start(out=hs[:], in_=hs_i[:])
        c82 = singles.tile([B, 2], F32)
        nc.sync.dma_start(out=c82[:], in_=c82_i[:])
        w8 = singles.tile([B, C], F32)
        nc.sync.dma_start(out=w8[:], in_=w_i[:].to_broadcast([B, C]))
        b8 = singles.tile([B, C], F32)
        nc.sync.dma_start(out=b8[:], in_=b_i[:].to_broadcast([B, C]))
        ones18 = singles.tile([1, B], F32)
        nc.vector.memset(ones18[:], 1.0)

        # ---------------- pass 1: per-sub-slice sums ----------------
        # bf16 partials: one rounding of an fp32-internal sum; relative error
        # ~0.4% on var / ~1e-5 absolute on means, well inside tolerance
        ctx.enter_context(nc.allow_low_precision("bf16 partial sums"))
        xs = []
        # S cols 0..7: per chunk-pair sum(x); cols 8..23: per-slice sum(x^2)
        S = singles.tile([P, NSC], BF16)
        d_ts = dpool.tile([P, SL], BF16, name="d_ts")
        d_stt = dpool.tile([P, SL], BF16, name="d_stt")
        d_act = dpool.tile([P, SL], BF16, name="d_act")
        for m in range(NSL):
            xm = xpool.tile([P, SL], BF16)
            eng = nc.sync if (m % 2 == 0) else nc.gpsimd
            eng.dma_start(out=xm[:], in_=xt_i[:, m * SL:(m + 1) * SL])
            xs.append(xm)
            if m in ACT_SQ:
                nc.scalar.activation(out=d_act[:], in_=xm[:],
                                     func=mybir.ActivationFunctionType.Square,
                                     accum_out=S[:, NCHH + m:NCHH + m + 1])
            else:
                nc.vector.scalar_tensor_tensor(
                    out=d_stt[:], in0=xm[:], scalar=1.0, in1=xm[:],
                    op0=mybir.AluOpType.mult, op1=mybir.AluOpType.mult,
                    accum_out=S[:, NCHH + m:NCHH + m + 1])
            if m % 2 == 1:
                # sum(x) over the whole chunk pair (slices m-1, m) in one op
                nc.vector.scalar_tensor_tensor(
                    out=d_ts[:], in0=xs[m - 1][:], scalar=1.0, in1=xm[:],
                    op0=mybir.AluOpType.mult, op1=mybir.AluOpType.add,
                    accum_out=S[:, m // 2:m // 2 + 1])

        # ---------------- aggregate chunk sums -> segment sums ----------------
        ps_t = psum.tile([NSC, P], BF16)
        nc.tensor.transpose(out=ps_t[:], in_=S[:], identity=identb[:])
        S_T = singles.tile([NSC, P], BF16)
        nc.vector.tensor_copy(out=S_T[:], in_=ps_t[:])

        ps_seg = psum.tile([16, C], F32)
        nc.tensor.matmul(out=ps_seg[:], lhsT=selagg[:, 0:16],
                         rhs=S_T[:, 0:C], start=True, stop=False)
        nc.tensor.matmul(out=ps_seg[:], lhsT=selagg[:, 16:32],
                         rhs=S_T[:, C:2 * C], start=False, stop=True)
        sseg = singles.tile([16, C], F32)
        nc.vector.tensor_copy(out=sseg[:], in_=ps_seg[:])

        # ---------------- all-reduce partials ----------------
        nc.scalar.dma_start(out=cc_in[:], in_=sseg[:])
        nc.gpsimd.collective_compute(
            "AllReduce", mybir.AluOpType.add,
            replica_groups=[list(range(NCORES))],
            ins=[cc_in[:]], outs=[cc_out[:]])
        s16 = singles.tile([16, C], F32)
        nc.scalar.dma_start(out=s16[:], in_=cc_out[:])

        # reshape [16, C] -> s12 [B, 2C] via two tiny matmuls
        ps2 = psum.tile([B, 2 * C], F32)
        nc.tensor.matmul(out=ps2[:, 0:C], lhsT=ident[0:16, 0:B],
                         rhs=s16[:], start=True, stop=True)
        nc.tensor.matmul(out=ps2[:, C:2 * C], lhsT=ident[0:16, B:16],
                         rhs=s16[:], start=True, stop=True)
        s12 = singles.tile([B, 2 * C], F32)
        nc.vector.tensor_copy(out=s12[:], in_=ps2[:])

        # ---------------- stats -> A/D tables ----------------
        invc = hs[:, 0:1]

        # mie = [mean_in | E2] in one fused op
        mie = singles.tile([B, 2 * C], F32)
        nc.vector.tensor_scalar(out=mie[:], in0=s12[:], scalar1=invc,
                                scalar2=None, op0=mybir.AluOpType.mult)
        mean_in = mie[:, 0:C]
        E2 = mie[:, C:2 * C]
        var_in = singles.tile([B, C], F32)
        nc.vector.tensor_tensor(out=var_in[:], in0=mean_in, in1=mean_in,
                                op=mybir.AluOpType.mult)
        nc.vector.tensor_tensor(out=var_in[:], in0=E2, in1=var_in[:],
                                op=mybir.AluOpType.subtract)

        # ml2 = [sum(mean_in), sum(E2)] / C  -> [mean_ln | E2_ln]
        ml2 = singles.tile([B, 2], F32)
        nc.vector.reduce_sum(out=ml2[:],
                             in_=mie[:].rearrange("p (a c) -> p a c", c=C),
                             axis=mybir.AxisListType.X)
        nc.vector.tensor_scalar(out=ml2[:], in0=ml2[:], scalar1=1.0 / C,
                                scalar2=None, op0=mybir.AluOpType.mult)
        mean_ln = ml2[:, 0:1]
        E2_ln = ml2[:, 1:2]
        var_ln = singles.tile([B, 1], F32)
        nc.vector.tensor_tensor(out=var_ln[:], in0=mean_ln, in1=mean_ln,
                                op=mybir.AluOpType.mult)
        nc.vector.tensor_tensor(out=var_ln[:], in0=E2_ln, in1=var_ln[:],
                                op=mybir.AluOpType.subtract)

        # column sums over segments (M=1 matmuls, results on partition 0)
        ps_cs = psum.tile([1, 4 * C], F32)
        nc.tensor.matmul(out=ps_cs[:, 0:2 * C], lhsT=c82[:, 0:1], rhs=s12[:],
                         start=True, stop=True)
        nc.tensor.matmul(out=ps_cs[:, 2 * C:4 * C], lhsT=c82[:, 1:2],
                         rhs=s12[:], start=True, stop=True)
        cs1 = singles.tile([1, 2 * C], F32)
        nc.vector.tensor_copy(out=cs1[:], in_=ps_cs[:, 0:2 * C])
        cs2 = singles.tile([1, 2 * C], F32)
        nc.vector.tensor_copy(out=cs2[:], in_=ps_cs[:, 2 * C:4 * C])
        # mean_bn = cs1[0, 0:C] ;  S2/(N-1) = cs2[0, C:2C]
        mvbn = singles.tile([1, 2 * C], F32)
        nc.vector.tensor_copy(out=mvbn[:, 0:C], in_=cs1[:, 0:C])
        mbn2 = singles.tile([1, C], F32)
        nc.vector.tensor_tensor(out=mbn2[:], in0=cs1[:, 0:C],
                                in1=cs1[:, 0:C], op=mybir.AluOpType.mult)
        nc.vector.tensor_scalar(out=mbn2[:], in0=mbn2[:],
                                scalar1=float(NTOT) / float(NTOT - 1),
                                scalar2=None, op0=mybir.AluOpType.mult)
        nc.vector.tensor_tensor(out=mvbn[:, C:2 * C], in0=cs2[:, C:2 * C],
                                in1=mbn2[:], op=mybir.AluOpType.subtract)

        # broadcast [1,128] -> [8,128] via K=1 matmul with ones
        ps_bc = psum.tile([B, 2 * C], F32)
        nc.tensor.matmul(out=ps_bc[:], lhsT=ones18[:], rhs=mvbn[:],
                         start=True, stop=True)
        bc = singles.tile([B, 2 * C], F32)
        nc.vector.tensor_copy(out=bc[:], in_=ps_bc[:])

        # mean = mw0*mean_in + mw1*mean_ln + mw2*mean_bn
        mls = singles.tile([B, 1], F32)
        nc.vector.tensor_tensor(out=mls[:], in0=mean_ln, in1=hs[:, 2:3],
                                op=mybir.AluOpType.mult)
        mean = singles.tile([B, C], F32)
        nc.vector.tensor_scalar(out=mean[:], in0=mean_in,
                                scalar1=hs[:, 1:2], scalar2=mls[:],
                                op0=mybir.AluOpType.mult,
                                op1=mybir.AluOpType.add)
        t2 = singles.tile([B, C], F32)
        nc.vector.tensor_scalar(out=t2[:], in0=bc[:, 0:C], scalar1=hs[:, 3:4],
                                scalar2=None, op0=mybir.AluOpType.mult)
        nc.vector.tensor_tensor(out=mean[:], in0=mean[:], in1=t2[:],
                                op=mybir.AluOpType.add)

        # var = vw0*var_in + vw1*var_ln + vw2*var_bn
        vls = singles.tile([B, 1], F32)
        nc.vector.tensor_tensor(out=vls[:], in0=var_ln, in1=hs[:, 5:6],
                                op=mybir.AluOpType.mult)
        var = singles.tile([B, C], F32)
        nc.vector.tensor_scalar(out=var[:], in0=var_in[:],
                                scalar1=hs[:, 4:5], scalar2=vls[:],
                                op0=mybir.AluOpType.mult,
                                op1=mybir.AluOpType.add)
        nc.vector.tensor_scalar(out=t2[:], in0=bc[:, C:2 * C],
                                scalar1=hs[:, 6:7], scalar2=None,
                                op0=mybir.AluOpType.mult)
        nc.vector.tensor_tensor(out=var[:], in0=var[:], in1=t2[:],
                                op=mybir.AluOpType.add)

        # inv_std = 1/sqrt(var+eps);  A = inv_std*w ; D = b - mean*A
        istd = singles.tile([B, C], F32)
        nc.scalar.activation(out=istd[:], in_=var[:],
                             func=mybir.ActivationFunctionType.Sqrt,
                             bias=hs[:, 7:8], scale=1.0)
        nc.vector.reciprocal(out=istd[:], in_=istd[:])
        AD = singles.tile([B, 2 * C], F32)
        nc.vector.tensor_tensor(out=AD[:, 0:C], in0=istd[:], in1=w8[:],
                                op=mybir.AluOpType.mult)
        mA = singles.tile([B, C], F32)
        nc.vector.tensor_tensor(out=mA[:], in0=mean[:], in1=AD[:, 0:C],
                                op=mybir.AluOpType.mult)
        nc.vector.tensor_tensor(out=AD[:, C:2 * C], in0=b8[:], in1=mA[:],
                                op=mybir.AluOpType.subtract)

        # per-partition per-sub-slice A/D vectors: ADt [128, 32]
        ps_ad = psum.tile([P, 2 * NSL], F32)
        nc.tensor.matmul(out=ps_ad[0:C, 0:NSL], lhsT=AD[:, 0:C],
                         rhs=selad[:, 0:NSL], start=True, stop=True)
        nc.tensor.matmul(out=ps_ad[C:P, 0:NSL], lhsT=AD[:, 0:C],
                         rhs=selad[:, NSL:2 * NSL], start=True, stop=True)
        nc.tensor.matmul(out=ps_ad[0:C, NSL:2 * NSL], lhsT=AD[:, C:2 * C],
                         rhs=selad[:, 0:NSL], start=True, stop=True)
        nc.tensor.matmul(out=ps_ad[C:P, NSL:2 * NSL], lhsT=AD[:, C:2 * C],
                         rhs=selad[:, NSL:2 * NSL], start=True, stop=True)
        ADt = singles.tile([P, 2 * NSL], F32)
        nc.vector.tensor_copy(out=ADt[:], in_=ps_ad[:])

        # ---------------- pass 2: normalize ----------------
        for m in range(NSL):
            ob = opool.tile([P, SL], BF16)
            nc.vector.tensor_scalar(out=ob[:], in0=xs[m][:],
                                    scalar1=ADt[:, m:m + 1],
                                    scalar2=ADt[:, NSL + m:NSL + m + 1],
                                    op0=mybir.AluOpType.mult,
                                    op1=mybir.AluOpType.add)
            eng = nc.gpsimd if (m % 2 == 0) else nc.scalar
            eng.dma_start(out=out_o[:, m * SL:(m + 1) * SL], in_=ob[:])

    nc.compile()
    return nc


def _get_nc():
    if "nc" not in _CACHE:
        _CACHE["nc"] = _build()
    return _CACHE["nc"]


def _softmax32(v):
    v = np.asarray(v, np.float32)
    e = np.exp(v - v.max())
    return (e / e.sum()).astype(np.float32)


def _prep_inputs(x, batch_ids, weight, bias, mean_weight, var_weight):
    x = np.asarray(x, np.float32)
    ids = np.asarray(batch_ids, np.int32)

    counts = np.bincount(ids, minlength=B)
    counts_c = np.maximum(counts, 1)
    mw = _softmax32(mean_weight)
    vw = _softmax32(var_weight)

    hs = np.zeros((B, 8), np.float32)
    hs[:, 0] = (1.0 / counts_c.astype(np.float64)).astype(np.float32)
    hs[:, 1] = mw[0]
    hs[:, 2] = mw[1]
    hs[:, 3] = mw[2]
    hs[:, 4] = vw[0]
    hs[:, 5] = vw[1]
    hs[:, 6] = vw[2]
    hs[:, 7] = EPS
    c82 = np.zeros((B, 2), np.float32)
    c82[:, 0] = 1.0 / NTOT
    c82[:, 1] = 1.0 / (NTOT - 1)
    wt = np.ascontiguousarray(np.asarray(weight, np.float32).reshape(1, C))
    bs = np.ascontiguousarray(np.asarray(bias, np.float32).reshape(1, C))
    ident = np.eye(P, dtype=np.float32)

    # --- sort points by segment; each 8192-point chunk single-segment ---
    order = np.argsort(ids, kind="stable")
    nchunks_b = (counts + CHP - 1) // CHP
    assert nchunks_b.sum() <= TOTCH, "segment sizes exceed chunk capacity"
    chunk_seg = np.full(TOTCH, -1, np.int64)
    seg_chunk_start = np.zeros(B + 1, np.int64)
    pos = 0
    for b in range(B):
        chunk_seg[pos:pos + nchunks_b[b]] = b
        seg_chunk_start[b] = pos
        pos += nchunks_b[b]
    seg_chunk_start[B] = pos

    cum = np.zeros(B + 1, np.int64)
    cum[1:] = np.cumsum(counts)
    ids_sorted = ids[order]
    within = np.arange(NTOT, dtype=np.int64) - cum[ids_sorted]
    dev_slot = seg_chunk_start[ids_sorted] * CHP + within

    xdev = np.zeros((NCORES * NLP, C), np.float32)
    xdev[dev_slot] = x[order]

    in_maps = []
    for i in range(NCORES):
        flat = xdev[i * NLP:(i + 1) * NLP]
        xt = np.ascontiguousarray(
            flat.reshape(2, HALF, C).transpose(0, 2, 1).reshape(P, HALF)
        ).astype(ml_dtypes.bfloat16)

        seg = chunk_seg[i * NCH:(i + 1) * NCH]   # 16 chunk segments, -1 unused
        # selagg rows: 0..7 chunk-pair sum(x) cols of S_T; 8..23 per-slice
        # sum(x^2) cols.  cols: 0:16 top-half lhsT block, 16:32 bottom.
        selagg = np.zeros((NSC, 32), np.float32)
        selad = np.zeros((B, 2 * NSL), np.float32)
        for j in range(NCHH):
            st, sb = seg[j], seg[NCHH + j]
            if st >= 0:
                selagg[j, st] = 1.0
            if sb >= 0:
                selagg[j, 16 + sb] = 1.0
        for m in range(NSL):
            st, sb = seg[m // 2], seg[NCHH + m // 2]
            if st >= 0:
                selagg[NCHH + m, B + st] = 1.0
                selad[st, m] = 1.0
            if sb >= 0:
                selagg[NCHH + m, 16 + B + sb] = 1.0
                selad[sb, NSL + m] = 1.0

        in_maps.append(dict(
            xt=xt, selagg=selagg.astype(ml_dtypes.bfloat16),
            identb=ident.astype(ml_dtypes.bfloat16),
            selad=selad, ident=ident,
            wt=wt, bs=bs, hs=hs, c82=c82))
    _CACHE["scatter"] = (order, dev_slot)
    return in_maps


def _postprocess(res):
    order, dev_slot = _CACHE["scatter"]
    flat = np.empty((NCORES * NLP, C), np.float32)
    for i in range(NCORES):
        o = np.asarray(res.results[i]["out"])
        flat[i * NLP:(i + 1) * NLP] = o.reshape(
            2, C, HALF).transpose(0, 2, 1).reshape(NLP, C).astype(np.float32)
    out = np.empty((NTOT, C), np.float32)
    out[order] = flat[dev_slot]
    return out


def kernel(x, batch_ids, weight, bias, mean_weight, var_weight):
    nc = _get_nc()
    in_maps = _prep_inputs(x, batch_ids, weight, bias,
                           mean_weight, var_weight)
    res = run_bass_kernel_spmd(nc, in_maps, list(range(NCORES)))
    _CACHE["last_result"] = res
    return _postprocess(res)


# revision 27
# speedup vs baseline: 1.6085x; 1.6085x over previous
"""MinkowskiSwitchNorm Trainium2 kernel (8 NeuronCores, Bass/Tile).

Strategy: host sorts points by segment id so that every 8192-point chunk is
single-segment, then ships x TRANSPOSED per core as [128, 65536] bf16 where
partition p = (half h = p//64, channel c = p%64) and free column j is point
h*65536 + j.  Free sub-slice m (width 4096) therefore lies inside chunk
m//2 (partitions 0:64) and chunk 8+m//2 (partitions 64:128).

Pass 1 computes per-(chunk,channel) sums: sum(x) via tensor_scalar+accum_out
on the vector engine, sum(x^2) via Square-activation+accum_out on the scalar
engine (with a few sub-slices on the vector engine's scalar_tensor_tensor to
balance load).  A PE transpose plus tiny one-hot selector matmuls aggregate
them into per-segment [2B, C] partials, combined across cores by a 4KB
AllReduce.  On-chip stats produce per-segment tables A = inv_std*w,
D = b - mean*A; four small matmuls expand them to per-partition per-slice
vectors, and pass 2 is a single fused tensor_scalar out = x*A + D per
sub-slice, written back as bf16 and upcast + unsorted on the host.  x stays
resident in SBUF between the passes, so HBM traffic is 16MB in + 16MB out
per core.
"""

import numpy as np
import ml_dtypes
from contextlib import ExitStack

import concourse.bass as bass
import concourse.tile as tile
from concourse import bacc, mybir
from concourse.bass_utils import run_bass_kernel_spmd

NCORES = 8
B = 8            # segments
C = 64           # channels
NTOT = 1_000_000
P = 128
NLP = 131072             # padded rows per core
HALF = NLP // 2          # 65536 free columns
CHP = 8192               # points per chunk
NCH = NLP // CHP         # chunks per core = 16
NSL = 16                 # free sub-slices per core
SL = HALF // NSL         # sub-slice width = 4096
TOTCH = NCORES * NCH     # 128 chunks globally
EPS = 1e-5
F32 = mybir.dt.float32
BF16 = mybir.dt.bfloat16

# engine assignment for sum(x^2) per sub-slice
ACT_SQ = frozenset([0, 1, 2, 3, 4, 5, 6, 7, 8, 9, 14, 15])  # rest: DVE STT
NCHH = NCH // 2          # chunk pairs per core = 8 (sum(x) granularity)
NSC = NCHH + NSL         # S columns: 8 chunk sum(x) + 16 slice sum(x^2)

_CACHE = {}


def _build():
    nc = bacc.Bacc("TRN2", target_bir_lowering=False, debug=False,
                   num_devices=NCORES)

    xt_i = nc.dram_tensor("xt", [P, HALF], BF16, kind="ExternalInput").ap()
    selagg_i = nc.dram_tensor("selagg", [NSC, 32], BF16,
                              kind="ExternalInput").ap()
    identb_i = nc.dram_tensor("identb", [P, P], BF16,
                              kind="ExternalInput").ap()
    selad_i = nc.dram_tensor("selad", [B, 2 * NSL], F32,
                             kind="ExternalInput").ap()
    ident_i = nc.dram_tensor("ident", [P, P], F32, kind="ExternalInput").ap()
    w_i = nc.dram_tensor("wt", [1, C], F32, kind="ExternalInput").ap()
    b_i = nc.dram_tensor("bs", [1, C], F32, kind="ExternalInput").ap()
    hs_i = nc.dram_tensor("hs", [B, 8], F32, kind="ExternalInput").ap()
    c82_i = nc.dram_tensor("c82", [B, 2], F32, kind="ExternalInput").ap()
    out_o = nc.dram_tensor("out", [P, HALF], BF16, kind="ExternalOutput").ap()

    cc_in = nc.dram_tensor("cc_in", [16, C], F32)
    cc_out = nc.dram_tensor("cc_out", [16, C], F32, addr_space="Shared")

    with ExitStack() as ctx:
        tc = ctx.enter_context(tile.TileContext(nc))
        xpool = ctx.enter_context(tc.tile_pool(name="xres", bufs=NSL))
        opool = ctx.enter_context(tc.tile_pool(name="outb", bufs=2))
        dpool = ctx.enter_context(tc.tile_pool(name="dumb", bufs=1))
        singles = ctx.enter_context(tc.tile_pool(name="singles", bufs=1))
        psum = ctx.enter_context(tc.tile_pool(name="ps", bufs=1, space="PSUM"))

        ident = singles.tile([P, P], F32)
        nc.sync.dma_start(out=ident[:], in_=ident_i[:])
        identb = singles.tile([P, P], BF16)
        nc.sync.dma_start(out=identb[:], in_=identb_i[:])
        selagg = singles.tile([NSC, 32], BF16)
        nc.sync.dma_start(out=selagg[:], in_=selagg_i[:])
        selad = singles.tile([B, 2 * NSL], F32)
        nc.sync.dma_start(out=selad[:], in_=selad_i[:])
        hs = singles.tile([B, 8], F32)
        nc.sync.dma_start(out=hs[:], in_=hs_i[:])
        c82 = singles.tile([B, 2], F32)
        nc.sync.dma_start(out=c82[:], in_=c82_i[:])
        w8 = singles.tile([B, C], F32)
        nc.sync.dma_start(out=w8[:], in_=w_i[:].to_broadcast([B, C]))
        b8 = singles.tile([B, C], F32)
        nc.sync.dma_start(out=b8[:], in_=b_i[:].to_broadcast([B, C]))
        ones18 = singles.tile([1, B], F32)
        nc.vector.memset(ones18[:], 1.0)

        # ---------------- pass 1: per-sub-slice sums ----------------
        # bf16 partials: one rounding of an fp32-internal sum; relative error
        # ~0.4% on var / ~1e-5 absolute on means, well inside tolerance
        ctx.enter_context(nc.allow_low_precision("bf16 partial sums"))
        xs = []
        # S cols 0..7: per chunk-pair sum(x); cols 8..23: per-slice sum(x^2)
        S = singles.tile([P, NSC], BF16)
        d_ts = dpool.tile([P, SL], BF16, name="d_ts")
        d_stt = dpool.tile([P, SL], BF16, name="d_stt")
        d_act = dpool.tile([P, SL], BF16, name="d_act")
        for m in range(NSL):
            xm = xpool.tile([P, SL], BF16)
            nc.sync.dma_start(out=xm[:], in_=xt_i[:, m * SL:(m + 1) * SL])
            xs.append(xm)
            if m in ACT_SQ:
                nc.scalar.activation(out=d_act[:], in_=xm[:],
                                     func=mybir.ActivationFunctionType.Square,
                                     accum_out=S[:, NCHH + m:NCHH + m + 1])
            else:
                nc.vector.scalar_tensor_tensor(
                    out=d_stt[:], in0=xm[:], scalar=1.0, in1=xm[:],
                    op0=mybir.AluOpType.mult, op1=mybir.AluOpType.mult,
                    accum_out=S[:, NCHH + m:NCHH + m + 1])
            if m % 2 == 1:
                # sum(x) over the whole chunk pair (slices m-1, m) in one op
                nc.vector.scalar_tensor_tensor(
                    out=d_ts[:], in0=xs[m - 1][:], scalar=1.0, in1=xm[:],
                    op0=mybir.AluOpType.mult, op1=mybir.AluOpType.add,
                    accum_out=S[:, m // 2:m // 2 + 1])

        # ---------------- aggregate chunk sums -> segment sums ----------------
        ps_t = psum.tile([NSC, P], BF16)
        nc.tensor.transpose(out=ps_t[:], in_=S[:], identity=identb[:])
        S_T = singles.tile([NSC, P], BF16)
        nc.vector.tensor_copy(out=S_T[:], in_=ps_t[:])

        ps_seg = psum.tile([16, C], F32)
        nc.tensor.matmul(out=ps_seg[:], lhsT=selagg[:, 0:16],
                         rhs=S_T[:, 0:C], start=True, stop=False)
        nc.tensor.matmul(out=ps_seg[:], lhsT=selagg[:, 16:32],
                         rhs=S_T[:, C:2 * C], start=False, stop=True)
        sseg = singles.tile([16, C], F32)
        nc.vector.tensor_copy(out=sseg[:], in_=ps_seg[:])

        # ---------------- all-reduce partials ----------------
        nc.scalar.dma_start(out=cc_in[:], in_=sseg[:])
        nc.gpsimd.collective_compute(
            "AllReduce", mybir.AluOpType.add,
            replica_groups=[list(range(NCORES))],
            ins=[cc_in[:]], outs=[cc_out[:]])
        s16 = singles.tile([16, C], F32)
        nc.scalar.dma_start(out=s16[:], in_=cc_out[:])

        # reshape [16, C] -> s12 [B, 2C] via two tiny matmuls
        ps2 = psum.tile([B, 2 * C], F32)
        nc.tensor.matmul(out=ps2[:, 0:C], lhsT=ident[0:16, 0:B],
                         rhs=s16[:], start=True, stop=True)
        nc.tensor.matmul(out=ps2[:, C:2 * C], lhsT=ident[0:16, B:16],
                         rhs=s16[:], start=True, stop=True)
        s12 = singles.tile([B, 2 * C], F32)
        nc.vector.tensor_copy(out=s12[:], in_=ps2[:])

        # ---------------- stats -> A/D tables ----------------
        invc = hs[:, 0:1]

        # mie = [mean_in | E2] in one fused op
        mie = singles.tile([B, 2 * C], F32)
        nc.vector.tensor_scalar(out=mie[:], in0=s12[:], scalar1=invc,
                                scalar2=None, op0=mybir.AluOpType.mult)
        mean_in = mie[:, 0:C]
        E2 = mie[:, C:2 * C]
        var_in = singles.tile([B, C], F32)
        nc.vector.tensor_tensor(out=var_in[:], in0=mean_in, in1=mean_in,
                                op=mybir.AluOpType.mult)
        nc.vector.tensor_tensor(out=var_in[:], in0=E2, in1=var_in[:],
                                op=mybir.AluOpType.subtract)

        # ml2 = [sum(mean_in), sum(E2)] / C  -> [mean_ln | E2_ln]
        ml2 = singles.tile([B, 2], F32)
        nc.vector.reduce_sum(out=ml2[:],
                             in_=mie[:].rearrange("p (a c) -> p a c", c=C),
                             axis=mybir.AxisListType.X)
        nc.vector.tensor_scalar(out=ml2[:], in0=ml2[:], scalar1=1.0 / C,
                                scalar2=None, op0=mybir.AluOpType.mult)
        mean_ln = ml2[:, 0:1]
        E2_ln = ml2[:, 1:2]
        var_ln = singles.tile([B, 1], F32)
        nc.vector.tensor_tensor(out=var_ln[:], in0=mean_ln, in1=mean_ln,
                                op=mybir.AluOpType.mult)
        nc.vector.tensor_tensor(out=var_ln[:], in0=E2_ln, in1=var_ln[:],
                                op=mybir.AluOpType.subtract)

        # column sums over segments (M=1 matmuls, results on partition 0)
        ps_cs = psum.tile([1, 4 * C], F32)
        nc.tensor.matmul(out=ps_cs[:, 0:2 * C], lhsT=c82[:, 0:1], rhs=s12[:],
                         start=True, stop=True)
        nc.tensor.matmul(out=ps_cs[:, 2 * C:4 * C], lhsT=c82[:, 1:2],
                         rhs=s12[:], start=True, stop=True)
        cs1 = singles.tile([1, 2 * C], F32)
        nc.vector.tensor_copy(out=cs1[:], in_=ps_cs[:, 0:2 * C])
        cs2 = singles.tile([1, 2 * C], F32)
        nc.vector.tensor_copy(out=cs2[:], in_=ps_cs[:, 2 * C:4 * C])
        # mean_bn = cs1[0, 0:C] ;  S2/(N-1) = cs2[0, C:2C]
        mvbn = singles.tile([1, 2 * C], F32)
        nc.vector.tensor_copy(out=mvbn[:, 0:C], in_=cs1[:, 0:C])
        mbn2 = singles.tile([1, C], F32)
        nc.vector.tensor_tensor(out=mbn2[:], in0=cs1[:, 0:C],
                                in1=cs1[:, 0:C], op=mybir.AluOpType.mult)
        nc.vector.tensor_scalar(out=mbn2[:], in0=mbn2[:],
                                scalar1=float(NTOT) / float(NTOT - 1),
                                scalar2=None, op0=mybir.AluOpType.mult)
        nc.vector.tensor_tensor(out=mvbn[:, C:2 * C], in0=cs2[:, C:2 * C],
                                in1=mbn2[:], op=mybir.AluOpType.subtract)

        # broadcast [1,128] -> [8,128] via K=1 matmul with ones
        ps_bc = psum.tile([B, 2 * C], F32)
        nc.tensor.matmul(out=ps_bc[:], lhsT=ones18[:], rhs=mvbn[:],
                         start=True, stop=True)
        bc = singles.tile([B, 2 * C], F32)
        nc.vector.tensor_copy(out=bc[:], in_=ps_bc[:])

        # mean = mw0*mean_in + mw1*mean_ln + mw2*mean_bn
        mls = singles.tile([B, 1], F32)
        nc.vector.tensor_tensor(out=mls[:], in0=mean_ln, in1=hs[:, 2:3],
                                op=mybir.AluOpType.mult)
        mean = singles.tile([B, C], F32)
        nc.vector.tensor_scalar(out=mean[:], in0=mean_in,
                                scalar1=hs[:, 1:2], scalar2=mls[:],
                                op0=mybir.AluOpType.mult,
                                op1=mybir.AluOpType.add)
        t2 = singles.tile([B, C], F32)
        nc.vector.tensor_scalar(out=t2[:], in0=bc[:, 0:C], scalar1=hs[:, 3:4],
                                scalar2=None, op0=mybir.AluOpType.mult)
        nc.vector.tensor_tensor(out=mean[:], in0=mean[:], in1=t2[:],
                                op=mybir.AluOpType.add)

        # var = vw0*var_in + vw1*var_ln + vw2*var_bn
        vls = singles.tile([B, 1], F32)
        nc.vector.tensor_tensor(out=vls[:], in0=var_ln, in1=hs[:, 5:6],
                                op=mybir.AluOpType.mult)
        var = singles.tile([B, C], F32)
        nc.vector.tensor_scalar(out=var[:], in0=var_in[:],
                                scalar1=hs[:, 4:5], scalar2=vls[:],
                                op0=mybir.AluOpType.mult,
                                op1=mybir.AluOpType.add)
        nc.vector.tensor_scalar(out=t2[:], in0=bc[:, C:2 * C],
                                scalar1=hs[:, 6:7], scalar2=None,
                                op0=mybir.AluOpType.mult)
        nc.vector.tensor_tensor(out=var[:], in0=var[:], in1=t2[:],
                                op=mybir.AluOpType.add)

        # inv_std = 1/sqrt(var+eps);  A = inv_std*w ; D = b - mean*A
        istd = singles.tile([B, C], F32)
        nc.scalar.activation(out=istd[:], in_=var[:],
                             func=mybir.ActivationFunctionType.Sqrt,
                             bias=hs[:, 7:8], scale=1.0)
        nc.vector.reciprocal(out=istd[:], in_=istd[:])
        AD = singles.tile([B, 2 * C], F32)
        nc.vector.tensor_tensor(out=AD[:, 0:C], in0=istd[:], in1=w8[:],
                                op=mybir.AluOpType.mult)
        mA = singles.tile([B, C], F32)
        nc.vector.tensor_tensor(out=mA[:], in0=mean[:], in1=AD[:, 0:C],
                                op=mybir.AluOpType.mult)
        nc.vector.tensor_tensor(out=AD[:, C:2 * C], in0=b8[:], in1=mA[:],
                                op=mybir.AluOpType.subtract)

        # per-partition per-sub-slice A/D vectors: ADt [128, 32]
        ps_ad = psum.tile([P, 2 * NSL], F32)
        nc.tensor.matmul(out=ps_ad[0:C, 0:NSL], lhsT=AD[:, 0:C],
                         rhs=selad[:, 0:NSL], start=True, stop=True)
        nc.tensor.matmul(out=ps_ad[C:P, 0:NSL], lhsT=AD[:, 0:C],
                         rhs=selad[:, NSL:2 * NSL], start=True, stop=True)
        nc.tensor.matmul(out=ps_ad[0:C, NSL:2 * NSL], lhsT=AD[:, C:2 * C],
                         rhs=selad[:, 0:NSL], start=True, stop=True)
        nc.tensor.matmul(out=ps_ad[C:P, NSL:2 * NSL], lhsT=AD[:, C:2 * C],
                         rhs=selad[:, NSL:2 * NSL], start=True, stop=True)
        ADt = singles.tile([P, 2 * NSL], F32)
        nc.vector.tensor_copy(out=ADt[:], in_=ps_ad[:])

        # ---------------- pass 2: normalize ----------------
        for m in range(NSL):
            ob = opool.tile([P, SL], BF16)
            nc.vector.tensor_scalar(out=ob[:], in0=xs[m][:],
                                    scalar1=ADt[:, m:m + 1],
                                    scalar2=ADt[:, NSL + m:NSL + m + 1],
                                    op0=mybir.AluOpType.mult,
                                    op1=mybir.AluOpType.add)
            eng = nc.sync if (m % 2 == 0) else nc.scalar
            eng.dma_start(out=out_o[:, m * SL:(m + 1) * SL], in_=ob[:])

    nc.compile()
    return nc


def _get_nc():
    if "nc" not in _CACHE:
        _CACHE["nc"] = _build()
    return _CACHE["nc"]


def _softmax32(v):
    v = np.asarray(v, np.float32)
    e = np.exp(v - v.max())
    return (e / e.sum()).astype(np.float32)


def _prep_inputs(x, batch_ids, weight, bias, mean_weight, var_weight):
    x = np.asarray(x, np.float32)
    ids = np.asarray(batch_ids, np.int32)

    counts = np.bincount(ids, minlength=B)
    counts_c = np.maximum(counts, 1)
    mw = _softmax32(mean_weight)
    vw = _softmax32(var_weight)

    hs = np.zeros((B, 8), np.float32)
    hs[:, 0] = (1.0 / counts_c.astype(np.float64)).astype(np.float32)
    hs[:, 1] = mw[0]
    hs[:, 2] = mw[1]
    hs[:, 3] = mw[2]
    hs[:, 4] = vw[0]
    hs[:, 5] = vw[1]
    hs[:, 6] = vw[2]
    hs[:, 7] = EPS
    c82 = np.zeros((B, 2), np.float32)
    c82[:, 0] = 1.0 / NTOT
    c82[:, 1] = 1.0 / (NTOT - 1)
    wt = np.ascontiguousarray(np.asarray(weight, np.float32).reshape(1, C))
    bs = np.ascontiguousarray(np.asarray(bias, np.float32).reshape(1, C))
    ident = np.eye(P, dtype=np.float32)

    # --- sort points by segment; each 8192-point chunk single-segment ---
    order = np.argsort(ids, kind="stable")
    nchunks_b = (counts + CHP - 1) // CHP
    assert nchunks_b.sum() <= TOTCH, "segment sizes exceed chunk capacity"
    chunk_seg = np.full(TOTCH, -1, np.int64)
    seg_chunk_start = np.zeros(B + 1, np.int64)
    pos = 0
    for b in range(B):
        chunk_seg[pos:pos + nchunks_b[b]] = b
        seg_chunk_start[b] = pos
        pos += nchunks_b[b]
    seg_chunk_start[B] = pos

    cum = np.zeros(B + 1, np.int64)
    cum[1:] = np.cumsum(counts)
    ids_sorted = ids[order]
    within = np.arange(NTOT, dtype=np.int64) - cum[ids_sorted]
    dev_slot = seg_chunk_start[ids_sorted] * CHP + within

    xdev = np.zeros((NCORES * NLP, C), np.float32)
    xdev[dev_slot] = x[order]

    in_maps = []
    for i in range(NCORES):
        flat = xdev[i * NLP:(i + 1) * NLP]
        xt = np.ascontiguousarray(
            flat.reshape(2, HALF, C).transpose(0, 2, 1).reshape(P, HALF)
        ).astype(ml_dtypes.bfloat16)

        seg = chunk_seg[i * NCH:(i + 1) * NCH]   # 16 chunk segments, -1 unused
        # selagg rows: 0..7 chunk-pair sum(x) cols of S_T; 8..23 per-slice
        # sum(x^2) cols.  cols: 0:16 top-half lhsT block, 16:32 bottom.
        selagg = np.zeros((NSC, 32), np.float32)
        selad = np.zeros((B, 2 * NSL), np.float32)
        for j in range(NCHH):
            st, sb = seg[j], seg[NCHH + j]
            if st >= 0:
                selagg[j, st] = 1.0
            if sb >= 0:
                selagg[j, 16 + sb] = 1.0
        for m in range(NSL):
            st, sb = seg[m // 2], seg[NCHH + m // 2]
            if st >= 0:
                selagg[NCHH + m, B + st] = 1.0
                selad[st, m] = 1.0
            if sb >= 0:
                selagg[NCHH + m, 16 + B + sb] = 1.0
                selad[sb, NSL + m] = 1.0

        in_maps.append(dict(
            xt=xt, selagg=selagg.astype(ml_dtypes.bfloat16),
            identb=ident.astype(ml_dtypes.bfloat16),
            selad=selad, ident=ident,
            wt=wt, bs=bs, hs=hs, c82=c82))
    _CACHE["scatter"] = (order, dev_slot)
    return in_maps


def _postprocess(res):
    order, dev_slot = _CACHE["scatter"]
    flat = np.empty((NCORES * NLP, C), np.float32)
    for i in range(NCORES):
        o = np.asarray(res.results[i]["out"])
        flat[i * NLP:(i + 1) * NLP] = o.reshape(
            2, C, HALF).transpose(0, 2, 1).reshape(NLP, C).astype(np.float32)
    out = np.empty((NTOT, C), np.float32)
    out[order] = flat[dev_slot]
    return out


def kernel(x, batch_ids, weight, bias, mean_weight, var_weight):
    nc = _get_nc()
    in_maps = _prep_inputs(x, batch_ids, weight, bias,
                           mean_weight, var_weight)
    res = run_bass_kernel_spmd(nc, in_maps, list(range(NCORES)))
    _CACHE["last_result"] = res
    return _postprocess(res)
